# revision 1
# baseline (speedup 1.0000x reference)
"""Fused decoder-layer kernel for one TRN2 chip (8 NeuronCores).

Problem (B=2, S=2048, DIM=1024, H=16, DH=64, DFF=2048):
    h1 = MHA(q=de_x, k=de_x, v=de_x, mask)   (shared per-head weights Wq/Wk/Wv)
    h2 = MHA(q=en_x, k=en_x, v=h1,  None)
    y  = relu(h2 @ W1 + b1) @ W2 + b2

Sharding: core c = 4*b + g  (b = batch, g = head-group of 4 heads; g also
indexes the S/4 slice of rows this core runs the FFN on).

Layout strategy (all matmul operands bf16, PSUM f32):
  - host passes x^T [DIM, S] per batch; q^T/k^T [DH, S] projections as in a
    standard transposed flow; v [S, DH] with a ones-column appended so the
    PV matmul also produces softmax row sums.
  - scores are computed transposed ([s2, s1]); exp'd score tiles then feed
    the PV matmuls as *stationary* operands with v moving, so PV output is
    s-major ([s1, DH]) and costs half the streamed rows of the v-stationary
    orientation.  Normalization is one reciprocal + one broadcast-multiply
    per (head, tile) unit.
  - h1/h2 head-blocks are transposed back to feature-major with cheap PE
    transposes (identity matmul) where a feature-contraction follows.
Cross-core:
  - instead of AllGather(h1), each core computes partial v2 contributions
    for every core of its batch group from its own h1 slice and a 4-way
    ReduceScatter(add) delivers the summed v2 shard (2 halves, one per
    query tile, each hidden under remaining attention compute).
  - h2^T is exchanged with two 8-way AllToAll halves (one per head pair);
    foreign-batch copies are zeroed via a per-core zmask input and folded,
    exactly as in the classic 2-group trick (4-way AllToAll is not
    supported by the mesh).
  - FFN layer 1 runs in two passes (one per AllToAll half) so the second
    half's wire time hides under the first half's matmuls.
"""

import math

import numpy as np
import ml_dtypes

import concourse.bass as bass
import concourse.bacc as bacc
import concourse.mybir as mybir
import concourse.tile as tile
import concourse.bass_utils as bass_utils

B, S, DIM, H = 2, 2048, 1024, 16
DH = DIM // H            # 64
DFF = 2048
NEG = -1.0e9
N_CORES = 8
G = 4                    # cores per batch group == head-groups == s-groups
HPC = H // G             # heads per core = 4
SL = S // G              # FFN rows per core = 512
KC = DIM // 128          # 8 k-chunks of the model dim
NB = S // 128            # 16 key blocks
NT = S // 1024           # 2 query tiles of 1024
TB = 1024 // 128         # 8 s1-blocks per query tile
BF16 = mybir.dt.bfloat16
F32 = mybir.dt.float32
AF = mybir.ActivationFunctionType

_CACHE: dict = {}


def _mask_plan(mask: np.ndarray):
    """Classify each [1024 x 128] (s1, s2) block: 'N' no-op, 'M' apply."""
    plan = []
    for t in range(NT):
        row = []
        for blk in range(NB):
            sub = mask[t * 1024:(t + 1) * 1024, blk * 128:(blk + 1) * 128]
            row.append('N' if not sub.any() else 'M')
        plan.append(tuple(row))
    return tuple(plan)


def _build(plan):
    has_mask = any(c == 'M' for row in plan for c in row)
    nc = bacc.Bacc("TRN2", target_bir_lowering=False, debug=False,
                   num_devices=N_CORES)

    de_xT = nc.dram_tensor("de_xT", [DIM, S], BF16, kind="ExternalInput")
    en_xT = nc.dram_tensor("en_xT", [DIM, S], BF16, kind="ExternalInput")
    wq = nc.dram_tensor("wq", [DIM, HPC * DH], BF16, kind="ExternalInput")
    wk = nc.dram_tensor("wk", [DIM, HPC * DH], BF16, kind="ExternalInput")
    wv = nc.dram_tensor("wv", [DIM, HPC * DH], BF16, kind="ExternalInput")
    wv2 = nc.dram_tensor("wv2", [HPC * DH, DIM], BF16, kind="ExternalInput")
    ident = nc.dram_tensor("ident", [128, 128], BF16, kind="ExternalInput")
    w1 = nc.dram_tensor("w1", [DIM, DFF], BF16, kind="ExternalInput")
    w2 = nc.dram_tensor("w2", [DFF, DIM], BF16, kind="ExternalInput")
    b1t = nc.dram_tensor("b1t", [128, DFF // 128], F32, kind="ExternalInput")
    b2t = nc.dram_tensor("b2t", [128, DIM // 128], F32, kind="ExternalInput")
    zmask = nc.dram_tensor("zmask", [128, 2 * KC], F32, kind="ExternalInput")
    maskT = None
    if has_mask:
        maskT = nc.dram_tensor("maskT", [S, S], BF16, kind="ExternalInput")
    yT = nc.dram_tensor("yT", [DIM, SL], F32, kind="ExternalOutput")

    with tile.TileContext(nc) as tc:
        _trace(nc, tc, plan, de_xT, en_xT, wq, wk, wv, wv2, ident,
               w1, w2, b1t, b2t, zmask, maskT, yT)
    nc.compile()
    return nc, has_mask


def _trace(nc, tc, plan, de_xT, en_xT, wq, wk, wv, wv2, ident,
           w1, w2, b1t, b2t, zmask, maskT, yT):
    # Unified allocation stack: everything (pools and single tiles) must be
    # released in strict LIFO order before TileContext exits.
    stack = nc._tile_stack = []

    def _push(release_fn):
        ent = {"f": release_fn}
        stack.append(ent)
        def rel():
            assert ent["f"] is not None, "double release"
            ent["f"](); ent["f"] = None
        return rel

    def release_rest():
        for ent in reversed(stack):
            if ent["f"] is not None:
                ent["f"](); ent["f"] = None

    def pool(**kw):
        cm = tc.tile_pool(**kw)
        p = cm.__enter__()
        _push(lambda: cm.__exit__(None, None, None))
        return p

    def single(shape, dtype, name):
        t_, f_ = tc.tile(shape, dtype, name=name)
        return t_, _push(f_)

    # ---- pools (creation order = reverse release order) -----------------
    dram = pool(name="dram", bufs=1, space="DRAM")
    ps = pool(name="ps", bufs=2, space="PSUM")       # [128,1024] f32: 4 banks
    pv = pool(name="pv", bufs=2, space="PSUM")       # [128,520] f32: 4 banks
    rc_pool = pool(name="rc", bufs=3)
    y_pool = pool(name="yp", bufs=2)
    qk_pool = pool(name="qk", bufs=2)                # q/k pair tiles
    v_pool = pool(name="vp", bufs=1)                 # v_all (attn1, attn2)
    hb_pool = pool(name="hb", bufs=2)                # h-big s-major tiles
    ht_pool = pool(name="ht", bufs=2)                # h^T feature-major tiles
    pd_pool = pool(name="pd", bufs=2)                # v2-partial staging
    pt_pool = pool(name="pt", bufs=6)                # exp'd score tiles

    # ---- persistent singles; en below de so de can be freed first -------
    wq_sb, _ = single([128, KC, HPC * DH], BF16, "wqsb")
    wk_sb, _ = single([128, KC, HPC * DH], BF16, "wksb")
    wv_sb, _ = single([128, KC, HPC * DH], BF16, "wvsb")
    wv2_sb, _ = single([128, 2, DIM], BF16, "wv2sb")
    id_sb, _ = single([128, 128], BF16, "idsb")
    b1_sb, _ = single([128, DFF // 128], F32, "b1sb")
    b2_sb, _ = single([128, DIM // 128], F32, "b2sb")
    zm_sb, _ = single([128, 2 * KC], F32, "zmsb")
    ones_sb, _ = single([128, 1], BF16, "onessb")
    en_sb, en_free = [], []
    for kc in range(KC):
        t_, f_ = single([128, S], BF16, f"en{kc}")
        en_sb.append(t_); en_free.append(f_)
    de_sb, de_free = [], []
    for kc in range(KC):
        t_, f_ = single([128, S], BF16, f"de{kc}")
        de_sb.append(t_); de_free.append(f_)

    # DMA trace order: attn1's operands first
    nc.sync.dma_start(wq_sb[:], wq.rearrange("(a p) c -> p a c", p=128))
    nc.sync.dma_start(wk_sb[:], wk.rearrange("(a p) c -> p a c", p=128))
    for kc in range(KC):
        nc.sync.dma_start(de_sb[kc][:], de_xT[kc * 128:(kc + 1) * 128, :])
    nc.sync.dma_start(wv_sb[:], wv.rearrange("(a p) c -> p a c", p=128))
    nc.sync.dma_start(id_sb[:], ident[:, :])
    nc.sync.dma_start(wv2_sb[:], wv2.rearrange("(a p) c -> p a c", p=128))
    for kc in range(KC):
        nc.sync.dma_start(en_sb[kc][:], en_xT[kc * 128:(kc + 1) * 128, :])
    nc.sync.dma_start(b1_sb[:], b1t[:])
    nc.sync.dma_start(b2_sb[:], b2t[:])
    nc.sync.dma_start(zm_sb[:], zmask[:])
    nc.vector.memset(ones_sb[:], 1.0)

    # collective bounce buffers (DRAM)
    rs1_in = [dram.tile([G * 1024, HPC * DH], BF16, name=f"rs1i{t}")
              for t in range(NT)]
    rs1_out = [dram.tile([1024, HPC * DH], BF16, name=f"rs1o{t}")
               for t in range(NT)]
    a2a_in = [dram.tile([KC * 128, SL], BF16, name=f"a2ai{p}")
              for p in range(2)]
    a2a_out = [dram.tile([KC * 128, SL], BF16, name=f"a2ao{p}")
               for p in range(2)]

    # ---- helpers --------------------------------------------------------
    def project_qk_pair(x_sb, w_sb, pair, tag):
        """q^T (or k^T) for one head pair as a [128, S] bf16 tile."""
        qt = qk_pool.tile([128, S], BF16, tag=tag, name=f"{tag}{pair}")
        for st in range(NT):
            psq = ps.tile([128, 1024], F32, tag="ps", name="pjps")
            for kc in range(KC):
                for nn in (0, 512):
                    nc.tensor.matmul(
                        psq[:, nn:nn + 512],
                        w_sb[:, kc, pair * 128:(pair + 1) * 128],
                        x_sb[kc][:, st * 1024 + nn:st * 1024 + nn + 512],
                        start=(kc == 0), stop=(kc == KC - 1))
            nc.vector.tensor_copy(qt[:, st * 1024:(st + 1) * 1024], psq[:])
        return qt

    def new_v_all(name):
        return v_pool.tile([128, NB * HPC * DH], BF16, tag="v", name=name)

    def project_v1(v_all):
        """v for 4 heads + ones column from de_x."""
        for blk in range(NB):
            psv = ps.tile([128, 1024], F32, tag="ps", name="vps")
            for kc in range(KC):
                nc.tensor.matmul(
                    psv[:, 0:HPC * DH],
                    de_sb[kc][:, blk * 128:(blk + 1) * 128],
                    wv_sb[:, kc, :],
                    start=(kc == 0), stop=(kc == KC - 1))
            nc.vector.tensor_copy(
                v_all[:, blk * HPC * DH:(blk + 1) * HPC * DH],
                psv[:, 0:HPC * DH])

    def attn_unit(q_pairs, k_pairs, v_all, aplan, mask_tiles_in, hbig, h, t):
        """scores -> exp -> PV (s1-major) -> normalize for one (head, tile).

        hbig: [128, TB*HPC*DH] s-major output tile for this t; head h's
        columns are the h*64 slice of each s1-block's 256-col group.
        """
        pair, off = h // 2, (h % 2) * 64
        pvh = pv.tile([128, TB * DH], F32, tag="pv", name="pvh")
        sums = pv.tile([128, TB], F32, tag="pvs", name="sums")
        pts = {}

        def score_exp(blk):
            sc = ps.tile([128, 1024], F32, tag="ps", name="scps")
            for nn in (0, 512):
                nc.tensor.matmul(
                    sc[:, nn:nn + 512],
                    k_pairs[pair][off:off + 64, blk * 128:(blk + 1) * 128],
                    q_pairs[pair][off:off + 64,
                                  t * 1024 + nn:t * 1024 + nn + 512],
                    start=True, stop=True)
            if aplan[t][blk] == 'M':
                nc.vector.tensor_add(sc[:], sc[:], mask_tiles_in[(t, blk)][:])
            pt = pt_pool.tile([128, 1024], BF16, name="pt")
            nc.scalar.activation(pt[:], sc[:], AF.Exp)
            pts[blk] = pt

        def pv_batch(blk):
            pt = pts.pop(blk)
            vs = v_all[:, (blk * HPC + h) * DH:(blk * HPC + h + 1) * DH]
            for sb in range(TB):
                first = blk == 0 and sb == 0
                last = blk == NB - 1 and sb == TB - 1
                nc.tensor.matmul(
                    pvh[:, sb * DH:(sb + 1) * DH],
                    pt[:, sb * 128:(sb + 1) * 128], vs,
                    start=first, stop=last)
                nc.tensor.matmul(
                    sums[:, sb:sb + 1],
                    pt[:, sb * 128:(sb + 1) * 128], ones_sb[:],
                    start=first, stop=last)

        # lag the PV consumption a few blocks behind the score producer so
        # the PE never queues a PV wait before independent score matmuls,
        # while keeping only ~LAG exp'd tiles live in the pt ring.
        LAG = 4
        for blk in range(NB):
            score_exp(blk)
            if blk >= LAG:
                pv_batch(blk - LAG)
        for blk in range(NB - LAG, NB):
            pv_batch(blk)
        # normalize: recip of the sums, per-block scalar multiply into hbig
        rec = rc_pool.tile([128, TB], F32, name="rec")
        nc.vector.reciprocal(rec[:], sums[:])
        h3 = hbig[:].rearrange("p (b c) -> p b c", c=HPC * DH)
        for sb in range(TB):
            nc.vector.tensor_scalar_mul(
                h3[:, sb, h * DH:(h + 1) * DH],
                pvh[:, sb * DH:(sb + 1) * DH], rec[:, sb:sb + 1])

    def transpose_block(src_ap, dst_tile, dst_lo):
        """PE-transpose one [128,128] block into dst_tile[:, dst_lo:+128]."""
        pst = pv.tile([128, 128], BF16, tag="pv", name="tpps")
        nc.tensor.transpose(pst[:], src_ap, id_sb[:])
        nc.vector.tensor_copy(dst_tile[:, dst_lo:dst_lo + 128], pst[:])

    # ---- attention 1 (self-attn on de_x, mask) --------------------------
    q1 = [None, None]
    k1 = [None, None]
    q1[0] = project_qk_pair(de_sb, wq_sb, 0, "q")
    k1[0] = project_qk_pair(de_sb, wk_sb, 0, "k")
    v1 = new_v_all("v1")
    project_v1(v1)
    mask_tiles, mask_free = {}, []
    for t in range(NT):
        for blk in range(NB):
            if plan[t][blk] == 'M':
                mt, fm = single([128, 1024], BF16, f"mk{t}_{blk}")
                nc.sync.dma_start(
                    mt[:], maskT[blk * 128:(blk + 1) * 128,
                                 t * 1024:(t + 1) * 1024])
                mask_tiles[(t, blk)] = mt
                mask_free.append(fm)

    # t-major so each query-tile half feeds its ReduceScatter while the
    # other half is still computing
    for t in range(NT):
        hbig = hb_pool.tile([128, TB * HPC * DH], BF16, tag="hb",
                            name=f"h1b{t}")
        for h in range(HPC):
            if t == 0 and h == 2:
                q1[1] = project_qk_pair(de_sb, wq_sb, 1, "q")
                k1[1] = project_qk_pair(de_sb, wk_sb, 1, "k")
            attn_unit(q1, k1, v1, plan, mask_tiles, hbig, h, t)
        # transpose this half of h1 to feature-major [256, 1024]
        h1T = [ht_pool.tile([128, 1024], BF16, tag="ht", name=f"h1T{t}_{fb}")
               for fb in range(2)]
        for sb in range(TB):
            for fb in range(2):
                transpose_block(
                    hbig[:, sb * 256 + fb * 128:sb * 256 + fb * 128 + 128],
                    h1T[fb], sb * 128)
        # partial v2 contributions for all 4 group members, then RS(add)
        for sb in range(TB):
            psd = ps.tile([128, 1024], F32, tag="ps", name="pdps")
            for fb in range(2):
                for nn in (0, 512):
                    nc.tensor.matmul(
                        psd[:, nn:nn + 512],
                        h1T[fb][:, sb * 128:(sb + 1) * 128],
                        wv2_sb[:, fb, nn:nn + 512],
                        start=(fb == 0), stop=(fb == 1))
            pd_sb = pd_pool.tile([128, 1024], BF16, tag="pd", name="pdsb")
            nc.vector.tensor_copy(pd_sb[:], psd[:])
            dst = rs1_in[t].rearrange("(d a p) c -> p d a c", d=G, a=TB)
            nc.sync.dma_start(
                dst[:, :, sb, :],
                pd_sb[:].rearrange("p (d c) -> p d c", d=G))
        nc.gpsimd.collective_compute(
            "ReduceScatter", mybir.AluOpType.add,
            replica_groups=[[0, 1, 2, 3], [4, 5, 6, 7]],
            ins=[rs1_in[t].opt()], outs=[rs1_out[t].opt()])
    for f in reversed(mask_free):
        f()
    for f in reversed(de_free):
        f()

    # ---- attention 2 (q,k from en_x; v from reduce-scattered h1@Wv) -----
    q2 = [None, None]
    k2 = [None, None]
    q2[0] = project_qk_pair(en_sb, wq_sb, 0, "q")
    k2[0] = project_qk_pair(en_sb, wk_sb, 0, "k")
    q2[1] = project_qk_pair(en_sb, wq_sb, 1, "q")
    k2[1] = project_qk_pair(en_sb, wk_sb, 1, "k")
    for f in reversed(en_free):
        f()

    # FFN weights prefetch during attention 2 (after en slabs are freed,
    # reusing their SBUF space — hence the pool is created only now)
    wpool = pool(name="wp", bufs=1)
    w1_sb = []
    for kc in range(KC):
        t_ = wpool.tile([128, DFF], BF16, tag="w1", bufs=KC, name=f"w1_{kc}")
        nc.sync.dma_start(t_[:], w1[kc * 128:(kc + 1) * 128, :])
        w1_sb.append(t_)
    w2_sb = []
    for dc in range(DFF // 128):
        t_ = wpool.tile([128, DIM], BF16, tag="w2", bufs=DFF // 128,
                        name=f"w2_{dc}")
        nc.sync.dma_start(t_[:], w2[dc * 128:(dc + 1) * 128, :])
        w2_sb.append(t_)

    # v2 arrives via the two ReduceScatters; strided DMA drops the 64-col
    # head groups straight into the 65-stride v_all layout
    v2 = new_v_all("v2")
    v23 = v2[:].rearrange("p (t b c) -> p t b c", t=NT, b=TB)
    for t in range(NT):
        nc.sync.dma_start(
            v23[:, t, :, :],
            rs1_out[t].rearrange("(b p) c -> p b c", p=128))

    # pair-major so each head-pair half of h2^T AllToAlls while the other
    # pair is still computing
    h2big = [hb_pool.tile([128, TB * HPC * DH], BF16, tag="hb",
                          name=f"h2b{t}") for t in range(NT)]
    noplan = tuple(tuple('N' for _ in range(NB)) for _ in range(NT))
    for pair in range(2):
        for hh in range(2):
            for t in range(NT):
                attn_unit(q2, k2, v2, noplan, {}, h2big[t], pair * 2 + hh, t)
        h2T = ht_pool.tile([128, S], BF16, tag="ht2", bufs=2,
                           name=f"h2T{pair}")
        for t in range(NT):
            for sb in range(TB):
                transpose_block(
                    h2big[t][:, sb * 256 + pair * 128:
                             sb * 256 + pair * 128 + 128],
                    h2T, t * 1024 + sb * 128)
        for half in range(2):
            nc.sync.dma_start(
                a2a_in[pair].rearrange(
                    "(j p) c -> p j c", p=128)[:, half * G:(half + 1) * G, :],
                h2T[:].rearrange("p (j c) -> p j c", c=SL))
        nc.gpsimd.collective_compute(
            "AllToAll", mybir.AluOpType.bypass,
            replica_groups=[[0, 1, 2, 3, 4, 5, 6, 7]],
            ins=[a2a_in[pair].opt()], outs=[a2a_out[pair].opt()])

    # ---- FFN on own S/4 rows -------------------------------------------
    # The A2A delivers each feature row twice (once per batch group); zero
    # the foreign-batch copy via the zmask input and fold the two copies.
    h2f = []                      # [pair][i] -> [128, SL] chunk APs
    for pair in range(2):
        f8 = wpool.tile([128, 2 * G, SL], BF16, tag="h2f", bufs=2,
                        name=f"h2f{pair}")
        nc.sync.dma_start(
            f8[:], a2a_out[pair].rearrange("(j p) c -> p j c", p=128))
        for j in range(2 * G):
            nc.vector.tensor_scalar_mul(
                f8[:, j, :], f8[:, j, :],
                zm_sb[:, pair * 2 * G + j:pair * 2 * G + j + 1])
        nc.vector.tensor_tensor(
            f8[:, 0:G, :], f8[:, 0:G, :], f8[:, G:2 * G, :],
            mybir.AluOpType.add)
        h2f.append([f8[:, i, :] for i in range(G)])

    # pass A: pair-0 partial sums land in SBUF bf16 while A2A-b is in flight
    fp_sb = []
    for dffb in range(DFF // 128):
        psf = ps.tile([128, 1024], F32, tag="ps", name="fAps")
        for i in range(G):
            nc.tensor.matmul(
                psf[:, 0:SL],
                w1_sb[2 * i][:, dffb * 128:(dffb + 1) * 128],
                h2f[0][i],
                start=(i == 0), stop=(i == G - 1))
        t_ = wpool.tile([128, SL], BF16, tag="fp", bufs=DFF // 128,
                        name=f"fp_{dffb}")
        nc.vector.tensor_copy(t_[:], psf[:, 0:SL])
        fp_sb.append(t_)
    # pass B: pair-1 contribution + pass-A partial, relu, bias
    ff1_sb = []
    for dffb in range(DFF // 128):
        psf = ps.tile([128, 1024], F32, tag="ps", name="fBps")
        for i in range(G):
            nc.tensor.matmul(
                psf[:, 0:SL],
                w1_sb[2 * i + 1][:, dffb * 128:(dffb + 1) * 128],
                h2f[1][i],
                start=(i == 0), stop=(i == G - 1))
        nc.vector.tensor_add(psf[:, 0:SL], psf[:, 0:SL], fp_sb[dffb][:])
        t_ = wpool.tile([128, SL], BF16, tag="ff1", bufs=DFF // 128,
                        name=f"ff1_{dffb}")
        nc.scalar.activation(t_[:], psf[:, 0:SL], AF.Relu,
                             bias=b1_sb[:, dffb:dffb + 1])
        ff1_sb.append(t_)
    for dimb in range(DIM // 128):
        psy = ps.tile([128, 1024], F32, tag="ps", name="yps")
        for dc in range(DFF // 128):
            nc.tensor.matmul(
                psy[:, 0:SL],
                w2_sb[dc][:, dimb * 128:(dimb + 1) * 128],
                ff1_sb[dc][:],
                start=(dc == 0), stop=(dc == DFF // 128 - 1))
        ysb = y_pool.tile([128, SL], F32, tag="y", name="ysb")
        nc.vector.tensor_scalar_add(ysb[:], psy[:, 0:SL],
                                    b2_sb[:, dimb:dimb + 1])
        nc.sync.dma_start(yT[dimb * 128:(dimb + 1) * 128, :], ysb[:])

    release_rest()


def _prep_inputs(de_x, en_x, mask, Wq, Wk, Wv, W1, b1, W2, b2, has_mask):
    bf = ml_dtypes.bfloat16
    scale = 1.0 / math.sqrt(DH)
    in_maps = []
    deT = [np.ascontiguousarray(de_x[b].T).astype(bf) for b in range(B)]
    enT = [np.ascontiguousarray(en_x[b].T).astype(bf) for b in range(B)]
    w1b = W1.astype(bf)
    w2b = W2.astype(bf)
    b1t = np.ascontiguousarray(b1.reshape(DFF // 128, 128).T).astype(np.float32)
    b2t = np.ascontiguousarray(b2.reshape(DIM // 128, 128).T).astype(np.float32)
    wv_full = np.transpose(Wv, (1, 0, 2)).reshape(DIM, DIM)
    identb = np.eye(128, dtype=bf)
    mT = None
    if has_mask:
        mT = np.ascontiguousarray(mask.T * np.float32(NEG)).astype(bf)
    for c in range(N_CORES):
        b, g = divmod(c, G)
        hs = slice(g * HPC, (g + 1) * HPC)
        m = {
            "de_xT": deT[b],
            "en_xT": enT[b],
            "wq": np.ascontiguousarray(
                np.transpose(Wq[hs] * scale, (1, 0, 2)).reshape(DIM, HPC * DH)
            ).astype(bf),
            "wk": np.ascontiguousarray(
                np.transpose(Wk[hs], (1, 0, 2)).reshape(DIM, HPC * DH)).astype(bf),
            "wv": np.ascontiguousarray(
                wv_full[:, g * 256:(g + 1) * 256]).astype(bf),
            "wv2": np.ascontiguousarray(
                wv_full[g * 256:(g + 1) * 256, :]).astype(bf),
            "ident": identb,
            "w1": w1b, "w2": w2b, "b1t": b1t, "b2t": b2t,
        }
        zm = np.zeros((128, 2 * KC), np.float32)
        for pair in range(2):
            for i in range(2 * G):
                if i // G == b:
                    zm[:, pair * 2 * G + i] = 1.0
        m["zmask"] = zm
        if has_mask:
            m["maskT"] = mT
        in_maps.append(m)
    return in_maps


def get_program(mask):
    plan = _mask_plan(np.asarray(mask))
    if plan not in _CACHE:
        _CACHE[plan] = _build(plan)
    return _CACHE[plan]


_RUNNERS: dict = {}


def _fast_runner(nc):
    """Build (once) a cached jitted SPMD executor for this program.

    run_bass_kernel_spmd re-creates and re-traces its jax.jit closure on
    every call; caching the jitted shard_map shaves seconds of dispatch
    overhead off warm calls. Mirrors bass2jax.run_bass_via_pjrt.
    """
    import jax
    from jax.sharding import Mesh, PartitionSpec
    try:
        from jax.experimental.shard_map import shard_map
    except ImportError:
        from jax.shard_map import shard_map
    import concourse.mybir as _mb
    from concourse import bass2jax as b2j

    b2j.install_neuronx_cc_hook()
    partition_name = (nc.partition_id_tensor.name
                      if nc.partition_id_tensor else None)
    in_names, out_names, out_avals = [], [], []
    for alloc in nc.m.functions[0].allocations:
        if not isinstance(alloc, _mb.MemoryLocationSet):
            continue
        name = alloc.memorylocations[0].name
        if alloc.kind == "ExternalInput":
            if name != partition_name:
                in_names.append(name)
        elif alloc.kind == "ExternalOutput":
            out_names.append(name)
            out_avals.append(jax.core.ShapedArray(
                tuple(alloc.tensor_shape), _mb.dt.np(alloc.dtype)))
    n_params = len(in_names)
    n_outs = len(out_avals)
    all_names = in_names + out_names + ([partition_name] if partition_name else [])
    donate = tuple(range(n_params, n_params + n_outs))

    def _body(*args):
        operands = list(args)
        if partition_name is not None:
            operands.append(b2j.partition_id_tensor())
        return tuple(b2j._bass_exec_p.bind(
            *operands,
            out_avals=tuple(out_avals),
            in_names=tuple(all_names),
            out_names=tuple(out_names),
            lowering_input_output_aliases=(),
            sim_require_finite=True,
            sim_require_nnan=True,
            nc=nc,
        ))

    devices = jax.devices()[:N_CORES]
    mesh = Mesh(np.asarray(devices), ("core",))
    in_specs = (PartitionSpec("core"),) * (n_params + n_outs)
    out_specs = (PartitionSpec("core"),) * n_outs
    sharded = jax.jit(
        shard_map(_body, mesh=mesh, in_specs=in_specs, out_specs=out_specs,
                  check_rep=False),
        donate_argnums=donate, keep_unused=True)

    def runner(in_maps):
        concat_in = [np.concatenate([in_maps[c][nm] for c in range(N_CORES)],
                                    axis=0) for nm in in_names]
        zeros = [np.zeros((N_CORES * a.shape[0], *a.shape[1:]), a.dtype)
                 for a in out_avals]
        out_arrs = sharded(*concat_in, *zeros)
        return [
            {nm: np.asarray(out_arrs[i]).reshape(N_CORES, *out_avals[i].shape)[c]
             for i, nm in enumerate(out_names)}
            for c in range(N_CORES)
        ]

    return runner


def run(inputs, want_results=False, **run_kwargs):
    nc, has_mask = get_program(inputs["mask"])
    in_maps = _prep_inputs(
        inputs["de_x"], inputs["en_x"], inputs["mask"],
        inputs["Wq"], inputs["Wk"], inputs["Wv"],
        inputs["W1"], inputs["b1"], inputs["W2"], inputs["b2"], has_mask)
    results = None
    res = None
    if not run_kwargs:
        try:
            key = id(nc)
            if key not in _RUNNERS:
                _RUNNERS[key] = _fast_runner(nc)
            results = _RUNNERS[key](in_maps)
        except Exception:
            results = None
    if results is None:
        res = bass_utils.run_bass_kernel_spmd(
            nc, in_maps, core_ids=list(range(N_CORES)), **run_kwargs)
        results = res.results
    y = np.empty((B, S, DIM), np.float32)
    for c in range(N_CORES):
        b, g = divmod(c, G)
        y[b, g * SL:(g + 1) * SL, :] = results[c]["yT"].T
    return (y, res) if want_results else y


def kernel(**inputs) -> np.ndarray:
    return run({k: np.asarray(v) for k, v in inputs.items()})

